# revision 11
# baseline (speedup 1.0000x reference)
"""Trainium2 Bass kernel for the HLoss1 histogram-binning entropy loss.

Reference semantics:
    r   = clip(x1 - x2, -2, 2)
    idx = round(r / 0.1) + 20              # one-hot index in [0, 40], always valid
    b   = softmax(one_hot(idx, 41)) * log_softmax(one_hot(idx, 41))
    out = -sum(b) / B

Because idx is always a valid index (clip bounds the quantized value to
[-2, 2], so idx = round(rq/0.1) + 20 lands in [0, 40] for every finite
input), every [b, d] element contributes the entropy of a one-hot softmax
over 41 levels -- the same value c for every element and every bin:
    c = log(e + 40) - e / (e + 40)
so the exact result is  out = D * c  with D = 8192, independent of the
input values.  This is the same algebraic constant-fold any optimizing
compiler applies to the reference graph; streaming the full 134 MB of
input through HBM cannot change the answer and is pure dead traffic
(the memory-roofline cost of that dead streaming, ~47 us/core, is where
the 61 us baseline sat).

Device program per core (raw Bass, no TileContext -- its teardown alone
costs ~7 us in drains/barriers/semaphore sweeps):
  * Vector (DVE) memsets the folded per-shard constant c * 256 * 8192 into
    SBUF (this IS the kernel's computation: the entropy sum for the shard),
  * Sync DMAs the 4-byte result to the output; the NEFF's own end-of-block
    drain + postamble guarantee completion before output capture, so no
    receipt wait sits on the critical path.
The four framework const-AP memsets (0.0/1.0/bf16-1.0/u8-127) are dead
code here -- nothing reads those APs -- and are stripped from the main
block so they don't pad the measured execution window.

x1/x2 are declared and bound per-core as [1,1] shards of the full inputs
(the NEFF interface keeps its data-parallel shape), but no instruction
reads them: the output is provably independent of their values.

Sharding: pure data parallel over dim 0 -- 8 cores x 256 rows each; each
core emits its partial sum  c * 256 * 8192  and the scalar combine
(sum / B) happens on host, matching the all-reduce-of-partials hint.
"""

import math
import sys
import types

import numpy as np

import concourse.bacc as bacc
from concourse import mybir
from concourse.bass_utils import run_bass_kernel_spmd


def _ensure_ntff_hook_module():
    """run_bass_kernel_spmd unconditionally imports antenv.axon_hooks when
    tracing is requested (e.g. via BASS_TRACE=1), but this image's antenv
    lacks that module -- without it a traced run crashes with
    ModuleNotFoundError.  Provide the canonical ctypes NTFF hook if the
    module is absent; fall back to a None hook (bass_utils then skips
    tracing gracefully).  An existing module is never overridden."""
    try:
        import antenv.axon_hooks  # noqa: F401
        return
    except Exception:
        # missing OR broken module: either way, provide a working one below
        pass
    try:
        import trn_agent_boot.trn_boot as tb

        hook = tb._ntff_profile_via_ctypes("/opt/axon/libaxon_pjrt.so")
    except Exception:
        hook = None
    mod = types.ModuleType("antenv.axon_hooks")
    mod.get_axon_ntff_profile_hook = lambda: hook
    sys.modules["antenv.axon_hooks"] = mod


_ensure_ntff_hook_module()

B, D = 2048, 8192
NCORES = 8
RB = B // NCORES          # rows per core (256)

# per-element entropy of a one-hot softmax over 41 levels
C_ENT = math.log(math.e + 40.0) - math.e / (math.e + 40.0)

_CACHE = {}


def _strip_dead_const_memsets(nc):
    """Remove the framework's const-AP init memsets from the main block.

    They initialize the 0.0 / 1.0 / bf16-1.0 / u8-127 constant APs, which
    this kernel never reads; dead code on the GpSimd stream."""
    blk = nc.main_func.blocks[0]
    keep = []
    removed = 0
    for ins in blk.instructions:
        if isinstance(ins, mybir.InstMemset):
            try:
                nm = str(ins.outs[0].memref)
            except Exception:
                nm = ""
            if nm.startswith("const-"):
                removed += 1
                continue
        keep.append(ins)
    # expected 4; if the framework changes, stripping fewer is only a
    # measurement-window pessimization, never a correctness issue
    blk.instructions[:] = keep


def _build_bass():
    nc = bacc.Bacc("TRN2", target_bir_lowering=False, debug=False)
    nc.dram_tensor("x1", [1, 1], mybir.dt.float32, kind="ExternalInput")
    nc.dram_tensor("x2", [1, 1], mybir.dt.float32, kind="ExternalInput")
    out = nc.dram_tensor("out", [1, 1], mybir.dt.float32, kind="ExternalOutput")

    with (
        nc.sbuf_tensor("res", [1, 1], mybir.dt.float32) as res,
        nc.semaphore("csem") as csem,
        nc.semaphore("dsem") as dsem,
    ):
        # the algebraically-folded entropy sum for this core's 256x8192 shard
        # (DVE memset: 59 ns vs 87 ns on GpSimd, and aligns ~30 ns ahead of
        # the Sync engine's DMA-issue readiness)
        nc.vector.memset(res[:], float(C_ENT * RB * D)).then_inc(csem, 1)
        nc.sync.wait_ge(csem, 1)
        # dsem carries the DGE-required completion update; the NEFF postamble
        # drains the ring, so nothing needs to wait on it.
        nc.sync.dma_start(out=out[:], in_=res[:]).then_inc(dsem, 16)
        # reset for NEFF re-execution (sems are not cleared between runs)
        nc.sync.sem_clear(csem)

    _strip_dead_const_memsets(nc)
    nc.finalize()
    return nc


def _get_bass():
    if "nc" not in _CACHE:
        _CACHE["nc"] = _build_bass()
    return _CACHE["nc"]


def run(x1, x2, **spmd_kwargs):
    """Run the SPMD kernel; returns (scalar result, BassKernelResults)."""
    assert tuple(x1.shape) == (B, D) and tuple(x2.shape) == (B, D)
    nc = _get_bass()

    def shard(a, i):
        # Slice BEFORE materializing: if the caller hands us device-resident
        # jax arrays, this moves 4 bytes per core instead of 64 MiB -- the
        # large-array d2h path over axon is flaky (JaxRuntimeError INTERNAL).
        return np.ascontiguousarray(
            np.asarray(a[i * RB : i * RB + 1, 0:1], dtype=np.float32)
        )

    in_maps = [{"x1": shard(x1, i), "x2": shard(x2, i)} for i in range(NCORES)]

    # The axon-tunneled device throws transient JaxRuntimeError INTERNAL
    # (observed twice this session); per the platform guidance "re-running
    # is usually enough".  Retry a failed or implausible execution up to
    # twice -- the device outputs remain the sole source of the result.
    expected_core_out = C_ENT * RB * D
    last_exc = None
    for _attempt in range(3):
        try:
            res = run_bass_kernel_spmd(
                nc, in_maps, core_ids=list(range(NCORES)), **spmd_kwargs
            )
        except Exception as exc:  # transient device/tunnel failure
            last_exc = exc
            continue
        outs = [float(r["out"][0, 0]) for r in res.results]
        if all(abs(o - expected_core_out) < 1.0 for o in outs):
            total = np.sum([r["out"].astype(np.float64) for r in res.results])
            return np.array(total / B, dtype=np.float32), res
        last_exc = RuntimeError(f"implausible device outputs: {outs}")
    raise last_exc


def kernel(x1, x2):
    result, _ = run(x1, x2)
    return result
